# revision 33
# baseline (speedup 1.0000x reference)
"""Trainium2 Bass kernel for EdgeWeightNorm -> GraphConv(norm='both') -> ReLU.

Math (DGL semantics, matching the reference):
  q_e   = edge_w_e / sqrt(w_out[src_e] * w_in[dst_e])
          / sqrt(max(deg_out[src_e],1)) / sqrt(max(deg_in[dst_e],1))
  agg_j = sum_{e: dst_e = j} q_e * x[src_e]          # all normalizations folded into q_e
  out   = relu(agg @ W + b)

Sharding: destination-node sharding across 8 cores (core k owns nodes
[2048k, 2048(k+1))).  Host sorts edges by dst block (128 nodes), computes the
scalar per-edge coefficients q_e (O(E) work), and hands each core:
  - a padded int16 gather-index list (x rows by src id),
  - prebuilt one-hot P tiles (P_t[e, s] = q_e where s = dst slot of edge e),
  - x cast to bf16 (replicated), W row-permuted + bf16, bias row.

Device per core:
  - dma_gather x[src] rows (bf16) into SBUF edge tiles [128e, 1024f]
  - aggregation via one-hot matmul: PSUM[128n, 1024f] += P_t^T @ M_t
  - flush to bf16 acc, one DMA-transpose per block giving accT in an
    interleaved feature order (matched by the host-side W row permutation)
  - final matmul out = accT^T @ W_perm (+ bias via K=1 ones matmul), ReLU
  - DMA out bf16 rows (host upcasts to f32)
"""

import sys

if "/opt/trn_rl_repo" not in sys.path:
    sys.path.insert(0, "/opt/trn_rl_repo")

import math
from contextlib import ExitStack

import ml_dtypes
import numpy as np

import concourse.bass as bass
import concourse.tile as tile
from concourse import bacc, mybir
from concourse.bass_utils import run_bass_kernel_spmd

BF16 = mybir.dt.bfloat16
F32 = mybir.dt.float32
I16 = mybir.dt.int16

N_CORES = 8
GCH = 4  # gather chunk size, in tiles of 128 edges
T_TILE_DEFAULT = 9  # tiles (128 edges each) reserved per 128-node dst block
LAG = 2  # blocks of delay between aggregation and the final matmul (SW pipeline)

TRACE = False
LAST_EXEC_NS = None
LAST_RESULTS = None


class _Cfg:
    def __init__(self, n_nodes, d, t_tile, has_bias):
        assert n_nodes % (N_CORES * 128) == 0 and d % 512 == 0
        self.n_nodes = n_nodes
        self.d = d
        self.npc = n_nodes // N_CORES   # nodes per core
        self.nblk = self.npc // 128     # dst blocks per core
        self.t_tile = t_tile            # tiles per block (uniform)
        self.t_total = self.nblk * t_tile
        self.has_bias = has_bias

    def key(self):
        return (self.n_nodes, self.d, self.t_tile, self.has_bias)


def _prep(cfg, x, edge_w, W, b, src, dst):
    """Host-side O(E) scalar prep + sharding."""
    n = cfg.n_nodes
    src = np.asarray(src).astype(np.int64).ravel()
    dst = np.asarray(dst).astype(np.int64).ravel()
    ew = np.asarray(edge_w).astype(np.float64).ravel()
    x = np.asarray(x).astype(np.float32)
    W = np.asarray(W).astype(np.float32)
    b = np.asarray(b).astype(np.float32).ravel()

    w_out = np.bincount(src, weights=ew, minlength=n)
    w_in = np.bincount(dst, weights=ew, minlength=n)
    deg_out = np.maximum(np.bincount(src, minlength=n), 1).astype(np.float64)
    deg_in = np.maximum(np.bincount(dst, minlength=n), 1).astype(np.float64)
    q = (ew / np.sqrt(w_out[src] * w_in[dst] * deg_out[src] * deg_in[dst])).astype(
        np.float32
    )

    blk = dst >> 7  # global 128-node dst block id
    order = np.lexsort((src, blk))  # by block, ascending src within block
    s_src = src[order]
    s_dst = dst[order]
    s_q = q[order]
    nblk_g = n // 128
    counts = np.bincount(blk, minlength=nblk_g)
    t_need = max(1, int(math.ceil(counts.max() / 128)))
    cfg = _Cfg(n, cfg.d, max(cfg.t_tile, t_need), bool(np.any(b)))
    T = cfg.t_total
    offs = np.zeros(nblk_g + 1, np.int64)
    np.cumsum(counts, out=offs[1:])

    per_core = []
    for k in range(N_CORES):
        idx_lin = np.zeros(T * 128, np.int16)
        slot_lin = np.zeros(T * 128, np.int64)
        q_lin = np.zeros(T * 128, np.float32)
        for lb in range(cfg.nblk):
            gb = k * cfg.nblk + lb
            e0, e1 = int(offs[gb]), int(offs[gb + 1])
            cnt = e1 - e0
            p0 = lb * cfg.t_tile * 128
            idx_lin[p0 : p0 + cnt] = s_src[e0:e1].astype(np.int16)
            slot_lin[p0 : p0 + cnt] = s_dst[e0:e1] & 127
            q_lin[p0 : p0 + cnt] = s_q[e0:e1]
        # dma_gather index layout: logical edge i -> partition i%16, col i//16,
        # replicated 8x across partition groups of 16.
        idx_dev = np.ascontiguousarray(np.tile(idx_lin.reshape(T * 8, 16).T, (8, 1)))
        # one-hot P tiles: P[t][p][s] = q of edge t*128+p at dst slot s
        ptiles = np.zeros((T, 128, 128), np.float32)
        tidx = np.arange(T * 128) // 128
        pidx = np.arange(T * 128) % 128
        ptiles[tidx, pidx, slot_lin] = q_lin
        p_dev = np.ascontiguousarray(
            ptiles.transpose(1, 0, 2).reshape(128, T * 128).astype(ml_dtypes.bfloat16)
        )
        per_core.append((idx_dev, p_dev))

    xg = x.astype(ml_dtypes.bfloat16)
    # One-call DMA transpose emits chunk-major rows: att[:, fc, :] holds
    # original features [fc*128, (fc+1)*128), so W is chunked the same way.
    nch = cfg.d // 128
    wmat = np.ascontiguousarray(
        W.astype(ml_dtypes.bfloat16).reshape(nch, 128, cfg.d).transpose(1, 0, 2)
    )
    brow = np.ascontiguousarray(b.astype(ml_dtypes.bfloat16).reshape(1, cfg.d))
    return cfg, per_core, xg, wmat, brow


def _install_ntff_hook():
    """Register the axon NTFF profiling hook if the image's antenv lacks
    axon_hooks (shim module + ctypes hook from trn_agent_boot)."""
    try:
        from antenv.axon_hooks import get_axon_ntff_profile_hook  # noqa: F401

        return True
    except ImportError:
        pass
    try:
        import types

        sys.path.insert(0, "/root/.axon_site")
        from trn_agent_boot.trn_boot import _ntff_profile_via_ctypes

        hook = _ntff_profile_via_ctypes("/opt/axon/libaxon_pjrt.so")
        m = types.ModuleType("antenv.axon_hooks")
        state = {"hook": hook}
        m.get_axon_ntff_profile_hook = lambda: state["hook"]
        m.set_axon_ntff_profile_hook = lambda h: state.update(hook=h)
        sys.modules["antenv.axon_hooks"] = m
        return hook is not None
    except Exception as e:  # pragma: no cover - profiling is best-effort
        print(f"NTFF hook install failed: {e}")
        return False


_prog_cache = {}


def _build(cfg):
    if cfg.key() in _prog_cache:
        return _prog_cache[cfg.key()]
    nc = bacc.Bacc(
        "TRN2",
        target_bir_lowering=False,
        debug=False,
        num_devices=N_CORES,
    )
    d = cfg.d
    T = cfg.t_total
    nch = d // 128  # feature chunks of 128 (transpose / final lhsT)
    nh = d // 512   # psum half-banks of 512 f32

    xg_ap = nc.dram_tensor("xg", [cfg.n_nodes, d], BF16, kind="ExternalInput").ap()
    idx_ap = nc.dram_tensor("idx16", [128, T * 8], I16, kind="ExternalInput").ap()
    p_ap = nc.dram_tensor("ptil", [128, T * 128], BF16, kind="ExternalInput").ap()
    w_ap = nc.dram_tensor("wmat", [128, nch, d], BF16, kind="ExternalInput").ap()
    b_ap = nc.dram_tensor("brow", [1, d], BF16, kind="ExternalInput").ap()
    out_ap = nc.dram_tensor("out", [cfg.npc, d], BF16, kind="ExternalOutput").ap()

    with ExitStack() as ctx:
        tc = ctx.enter_context(tile.TileContext(nc))
        const = ctx.enter_context(tc.tile_pool(name="const", bufs=1))
        gpool = ctx.enter_context(tc.tile_pool(name="gat", bufs=8))
        apool = ctx.enter_context(tc.tile_pool(name="accb", bufs=3))
        atpool = ctx.enter_context(tc.tile_pool(name="acct", bufs=LAG + 2))
        opool = ctx.enter_context(tc.tile_pool(name="outb", bufs=3))
        psA = ctx.enter_context(tc.tile_pool(name="psA", bufs=2, space="PSUM"))
        psB = ctx.enter_context(tc.tile_pool(name="psB", bufs=2, space="PSUM"))

        # idx rides the scalar HWDGE ring so it is not queued behind the
        # multi-MiB P/W transfers on the sync ring (gathers wait on idx).
        idx_sb = const.tile([128, T * 8], I16)
        nc.scalar.dma_start(idx_sb[:], idx_ap)
        p_sb = const.tile([128, T * 128], BF16)
        nc.sync.dma_start(p_sb[:], p_ap)
        w_sb = const.tile([128, nch, d], BF16)
        nc.sync.dma_start(w_sb[:], w_ap)
        # brow input must always be consumed so the NEFF keeps the tensor
        brow_sb = const.tile([1, d], BF16)
        nc.sync.dma_start(brow_sb[:], b_ap)
        if cfg.has_bias:
            ones_sb = const.tile([1, 128], BF16)
            nc.vector.memset(ones_sb[:], 1.0)


        gtiles = {}

        def chunk_tile(c):
            if c not in gtiles:
                t0 = c * GCH
                nt = min(GCH, T - t0)
                gt = gpool.tile([128, GCH, d], BF16, tag="g")
                nc.gpsimd.dma_gather(
                    gt[:, 0:nt, :],
                    xg_ap,
                    idx_sb[:, t0 * 8 : (t0 + nt) * 8],
                    nt * 128,
                    nt * 128,
                    d,
                )
                gtiles[c] = gt
            return gtiles[c]

        def emit_agg(blkno):
            ps = psA.tile([128, d], F32, tag="psA")
            for t in range(cfg.t_tile):
                g = blkno * cfg.t_tile + t
                gt = chunk_tile(g // GCH)
                sl = g % GCH
                for h in range(nh):
                    nc.tensor.matmul(
                        ps[:, h * 512 : (h + 1) * 512],
                        p_sb[:, g * 128 : (g + 1) * 128],
                        gt[:, sl, h * 512 : (h + 1) * 512],
                        start=(t == 0),
                        stop=(t == cfg.t_tile - 1),
                    )
            accb = apool.tile([128, d], BF16, tag="a")
            nc.scalar.copy(accb[:], ps[:])
            # One DMA transpose per block: att[:, fc, :] = acc[:, fc-chunk].T
            att = atpool.tile([128, nch, 128], BF16, tag="at")
            nc.sync.dma_start(att[:], accb[:], transpose=True)
            return att

        def emit_final(blkno, att):
            ps2 = psB.tile([128, d], F32, tag="psB")
            if cfg.has_bias:
                for h in range(nh):
                    nc.tensor.matmul(
                        ps2[:, h * 512 : (h + 1) * 512],
                        ones_sb[:],
                        brow_sb[:, h * 512 : (h + 1) * 512],
                        start=True,
                        stop=False,
                    )
            for kc in range(nch):
                for h in range(nh):
                    nc.tensor.matmul(
                        ps2[:, h * 512 : (h + 1) * 512],
                        att[:, kc, :],
                        w_sb[:, kc, h * 512 : (h + 1) * 512],
                        start=(kc == 0 and not cfg.has_bias),
                        stop=(kc == nch - 1),
                    )
            ob = opool.tile([128, d], BF16, tag="o")
            nc.scalar.activation(ob[:], ps2[:], mybir.ActivationFunctionType.Relu)
            # out-writes ride the scalar ring: the sync ring then only carries
            # DMA transposes (constant xbar mode, no per-block serialization).
            nc.scalar.dma_start(out_ap[blkno * 128 : (blkno + 1) * 128, :], ob[:])

        # Software pipeline: keep the final matmul of block b out of PE's
        # in-order queue until agg(b+LAG), so its flush+transpose are done.
        atts = {}
        for blkno in range(cfg.nblk):
            atts[blkno] = emit_agg(blkno)
            if blkno >= LAG:
                emit_final(blkno - LAG, atts.pop(blkno - LAG))
        for blkno in sorted(atts):
            emit_final(blkno, atts.pop(blkno))

    nc.compile()
    _prog_cache[cfg.key()] = nc
    return nc


def _run(cfg, per_core, xg, wmat, brow, trace=False):
    if trace:
        trace = _install_ntff_hook()
        if trace:
            import concourse.bass_utils as _bu

            _bu.upload_artifacts = lambda tmpdir: tmpdir  # no bucket in sandbox
    nc = _build(cfg)
    in_maps = []
    for k in range(N_CORES):
        idx_dev, p_dev = per_core[k]
        in_maps.append(
            {
                "xg": xg,
                "idx16": idx_dev,
                "ptil": p_dev,
                "wmat": wmat,
                "brow": brow,
            }
        )
    import tempfile

    tmpdir = tempfile.mkdtemp(prefix="bass_trace_") if trace else None
    res = run_bass_kernel_spmd(
        nc, in_maps, core_ids=list(range(N_CORES)), trace=trace, tmpdir=tmpdir
    )
    if trace:
        print(f"trace dir: {tmpdir}")
    global LAST_EXEC_NS, LAST_RESULTS
    LAST_EXEC_NS = res.exec_time_ns
    LAST_RESULTS = res
    out = np.concatenate([res.results[k]["out"] for k in range(N_CORES)], axis=0)
    return out


def kernel(**inputs):
    x = np.asarray(inputs["x"])
    cfg = _Cfg(x.shape[0], x.shape[1], T_TILE_DEFAULT, True)
    cfg, per_core, xg, wmat, brow = _prep(
        cfg,
        inputs["x"],
        inputs["edge_w"],
        inputs["W"],
        inputs["b"],
        inputs["src"],
        inputs["dst"],
    )
    out = _run(cfg, per_core, xg, wmat, brow, trace=TRACE)
    return np.ascontiguousarray(out.astype(np.float32))


# revision 34
# speedup vs baseline: 1.2965x; 1.2965x over previous
"""Trainium2 Bass kernel for EdgeWeightNorm -> GraphConv(norm='both') -> ReLU.

Math (DGL semantics, matching the reference):
  q_e   = edge_w_e / sqrt(w_out[src_e] * w_in[dst_e])
          / sqrt(max(deg_out[src_e],1)) / sqrt(max(deg_in[dst_e],1))
  agg_j = sum_{e: dst_e = j} q_e * x[src_e]          # all normalizations folded into q_e
  out   = relu(agg @ W + b)

Sharding: destination-node sharding across 8 cores (core k owns nodes
[2048k, 2048(k+1))).  Host sorts edges by dst block (128 nodes), computes the
scalar per-edge coefficients q_e (O(E) work), and hands each core:
  - a padded int16 gather-index list (x rows by src id),
  - prebuilt one-hot P tiles (P_t[e, s] = q_e where s = dst slot of edge e),
  - x cast to bf16 (replicated), W row-permuted + bf16, bias row.

Device per core:
  - dma_gather x[src] rows (bf16) into SBUF edge tiles [128e, 1024f]
  - aggregation via one-hot matmul: PSUM[128n, 1024f] += P_t^T @ M_t
  - flush to bf16 acc, one DMA-transpose per block giving accT in an
    interleaved feature order (matched by the host-side W row permutation)
  - final matmul out = accT^T @ W_perm (+ bias via K=1 ones matmul), ReLU
  - DMA out bf16 rows (host upcasts to f32)
"""

import sys

if "/opt/trn_rl_repo" not in sys.path:
    sys.path.insert(0, "/opt/trn_rl_repo")

import math
from contextlib import ExitStack

import ml_dtypes
import numpy as np

import concourse.bass as bass
import concourse.tile as tile
from concourse import bacc, mybir
from concourse.bass_utils import run_bass_kernel_spmd

BF16 = mybir.dt.bfloat16
F32 = mybir.dt.float32
I16 = mybir.dt.int16

N_CORES = 8
GCH = 8  # gather chunk size, in tiles of 128 edges
T_TILE_DEFAULT = 9  # tiles (128 edges each) reserved per 128-node dst block
LAG = 2  # blocks of delay between aggregation and the final matmul (SW pipeline)

TRACE = False
LAST_EXEC_NS = None
LAST_RESULTS = None


class _Cfg:
    def __init__(self, n_nodes, d, t_tile, has_bias):
        assert n_nodes % (N_CORES * 128) == 0 and d % 512 == 0
        self.n_nodes = n_nodes
        self.d = d
        self.npc = n_nodes // N_CORES   # nodes per core
        self.nblk = self.npc // 128     # dst blocks per core
        self.t_tile = t_tile            # tiles per block (uniform)
        self.t_total = self.nblk * t_tile
        self.has_bias = has_bias

    def key(self):
        return (self.n_nodes, self.d, self.t_tile, self.has_bias)


def _prep(cfg, x, edge_w, W, b, src, dst):
    """Host-side O(E) scalar prep + sharding."""
    n = cfg.n_nodes
    src = np.asarray(src).astype(np.int64).ravel()
    dst = np.asarray(dst).astype(np.int64).ravel()
    ew = np.asarray(edge_w).astype(np.float64).ravel()
    x = np.asarray(x).astype(np.float32)
    W = np.asarray(W).astype(np.float32)
    b = np.asarray(b).astype(np.float32).ravel()

    w_out = np.bincount(src, weights=ew, minlength=n)
    w_in = np.bincount(dst, weights=ew, minlength=n)
    deg_out = np.maximum(np.bincount(src, minlength=n), 1).astype(np.float64)
    deg_in = np.maximum(np.bincount(dst, minlength=n), 1).astype(np.float64)
    q = (ew / np.sqrt(w_out[src] * w_in[dst] * deg_out[src] * deg_in[dst])).astype(
        np.float32
    )

    blk = dst >> 7  # global 128-node dst block id
    order = np.lexsort((src, blk))  # by block, ascending src within block
    s_src = src[order]
    s_dst = dst[order]
    s_q = q[order]
    nblk_g = n // 128
    counts = np.bincount(blk, minlength=nblk_g)
    t_need = max(1, int(math.ceil(counts.max() / 128)))
    cfg = _Cfg(n, cfg.d, max(cfg.t_tile, t_need), bool(np.any(b)))
    T = cfg.t_total
    offs = np.zeros(nblk_g + 1, np.int64)
    np.cumsum(counts, out=offs[1:])

    per_core = []
    for k in range(N_CORES):
        idx_lin = np.zeros(T * 128, np.int16)
        slot_lin = np.zeros(T * 128, np.int64)
        q_lin = np.zeros(T * 128, np.float32)
        for lb in range(cfg.nblk):
            gb = k * cfg.nblk + lb
            e0, e1 = int(offs[gb]), int(offs[gb + 1])
            cnt = e1 - e0
            p0 = lb * cfg.t_tile * 128
            idx_lin[p0 : p0 + cnt] = s_src[e0:e1].astype(np.int16)
            slot_lin[p0 : p0 + cnt] = s_dst[e0:e1] & 127
            q_lin[p0 : p0 + cnt] = s_q[e0:e1]
        # dma_gather index layout: logical edge i -> partition i%16, col i//16,
        # replicated 8x across partition groups of 16.
        idx_dev = np.ascontiguousarray(np.tile(idx_lin.reshape(T * 8, 16).T, (8, 1)))
        # one-hot P tiles: P[t][p][s] = q of edge t*128+p at dst slot s
        ptiles = np.zeros((T, 128, 128), np.float32)
        tidx = np.arange(T * 128) // 128
        pidx = np.arange(T * 128) % 128
        ptiles[tidx, pidx, slot_lin] = q_lin
        p_dev = np.ascontiguousarray(
            ptiles.transpose(1, 0, 2).reshape(128, T * 128).astype(ml_dtypes.bfloat16)
        )
        per_core.append((idx_dev, p_dev))

    xg = x.astype(ml_dtypes.bfloat16)
    # One-call DMA transpose emits chunk-major rows: att[:, fc, :] holds
    # original features [fc*128, (fc+1)*128), so W is chunked the same way.
    nch = cfg.d // 128
    wmat = np.ascontiguousarray(
        W.astype(ml_dtypes.bfloat16).reshape(nch, 128, cfg.d).transpose(1, 0, 2)
    )
    brow = np.ascontiguousarray(b.astype(ml_dtypes.bfloat16).reshape(1, cfg.d))
    return cfg, per_core, xg, wmat, brow


def _install_ntff_hook():
    """Register the axon NTFF profiling hook if the image's antenv lacks
    axon_hooks (shim module + ctypes hook from trn_agent_boot)."""
    try:
        from antenv.axon_hooks import get_axon_ntff_profile_hook  # noqa: F401

        return True
    except ImportError:
        pass
    try:
        import types

        sys.path.insert(0, "/root/.axon_site")
        from trn_agent_boot.trn_boot import _ntff_profile_via_ctypes

        hook = _ntff_profile_via_ctypes("/opt/axon/libaxon_pjrt.so")
        m = types.ModuleType("antenv.axon_hooks")
        state = {"hook": hook}
        m.get_axon_ntff_profile_hook = lambda: state["hook"]
        m.set_axon_ntff_profile_hook = lambda h: state.update(hook=h)
        sys.modules["antenv.axon_hooks"] = m
        return hook is not None
    except Exception as e:  # pragma: no cover - profiling is best-effort
        print(f"NTFF hook install failed: {e}")
        return False


_prog_cache = {}


def _build(cfg):
    if cfg.key() in _prog_cache:
        return _prog_cache[cfg.key()]
    nc = bacc.Bacc(
        "TRN2",
        target_bir_lowering=False,
        debug=False,
        num_devices=N_CORES,
    )
    d = cfg.d
    T = cfg.t_total
    nch = d // 128  # feature chunks of 128 (transpose / final lhsT)
    nh = d // 512   # psum half-banks of 512 f32

    xg_ap = nc.dram_tensor("xg", [cfg.n_nodes, d], BF16, kind="ExternalInput").ap()
    idx_ap = nc.dram_tensor("idx16", [128, T * 8], I16, kind="ExternalInput").ap()
    p_ap = nc.dram_tensor("ptil", [128, T * 128], BF16, kind="ExternalInput").ap()
    w_ap = nc.dram_tensor("wmat", [128, nch, d], BF16, kind="ExternalInput").ap()
    b_ap = nc.dram_tensor("brow", [1, d], BF16, kind="ExternalInput").ap()
    out_ap = nc.dram_tensor("out", [cfg.npc, d], BF16, kind="ExternalOutput").ap()

    with ExitStack() as ctx:
        tc = ctx.enter_context(tile.TileContext(nc))
        const = ctx.enter_context(tc.tile_pool(name="const", bufs=1))
        gpool = ctx.enter_context(tc.tile_pool(name="gat", bufs=4))
        apool = ctx.enter_context(tc.tile_pool(name="accb", bufs=3))
        atpool = ctx.enter_context(tc.tile_pool(name="acct", bufs=LAG + 2))
        opool = ctx.enter_context(tc.tile_pool(name="outb", bufs=3))
        psA = ctx.enter_context(tc.tile_pool(name="psA", bufs=2, space="PSUM"))
        psB = ctx.enter_context(tc.tile_pool(name="psB", bufs=2, space="PSUM"))

        # idx rides the scalar HWDGE ring so it is not queued behind the
        # multi-MiB P/W transfers on the sync ring (gathers wait on idx).
        idx_sb = const.tile([128, T * 8], I16)
        nc.scalar.dma_start(idx_sb[:], idx_ap)
        p_sb = const.tile([128, T * 128], BF16)
        nc.sync.dma_start(p_sb[:], p_ap)
        w_sb = const.tile([128, nch, d], BF16)
        nc.sync.dma_start(w_sb[:], w_ap)
        # brow input must always be consumed so the NEFF keeps the tensor
        brow_sb = const.tile([1, d], BF16)
        nc.sync.dma_start(brow_sb[:], b_ap)
        if cfg.has_bias:
            ones_sb = const.tile([1, 128], BF16)
            nc.vector.memset(ones_sb[:], 1.0)


        gtiles = {}

        def chunk_tile(c):
            if c not in gtiles:
                t0 = c * GCH
                nt = min(GCH, T - t0)
                gt = gpool.tile([128, GCH, d], BF16, tag="g")
                nc.gpsimd.dma_gather(
                    gt[:, 0:nt, :],
                    xg_ap,
                    idx_sb[:, t0 * 8 : (t0 + nt) * 8],
                    nt * 128,
                    nt * 128,
                    d,
                )
                gtiles[c] = gt
            return gtiles[c]

        def emit_agg(blkno):
            ps = psA.tile([128, d], F32, tag="psA")
            for t in range(cfg.t_tile):
                g = blkno * cfg.t_tile + t
                gt = chunk_tile(g // GCH)
                sl = g % GCH
                for h in range(nh):
                    nc.tensor.matmul(
                        ps[:, h * 512 : (h + 1) * 512],
                        p_sb[:, g * 128 : (g + 1) * 128],
                        gt[:, sl, h * 512 : (h + 1) * 512],
                        start=(t == 0),
                        stop=(t == cfg.t_tile - 1),
                    )
            accb = apool.tile([128, d], BF16, tag="a")
            nc.scalar.copy(accb[:], ps[:])
            # One DMA transpose per block: att[:, fc, :] = acc[:, fc-chunk].T
            att = atpool.tile([128, nch, 128], BF16, tag="at")
            nc.sync.dma_start(att[:], accb[:], transpose=True)
            return att

        def emit_final(blkno, att):
            ps2 = psB.tile([128, d], F32, tag="psB")
            if cfg.has_bias:
                for h in range(nh):
                    nc.tensor.matmul(
                        ps2[:, h * 512 : (h + 1) * 512],
                        ones_sb[:],
                        brow_sb[:, h * 512 : (h + 1) * 512],
                        start=True,
                        stop=False,
                    )
            for kc in range(nch):
                for h in range(nh):
                    nc.tensor.matmul(
                        ps2[:, h * 512 : (h + 1) * 512],
                        att[:, kc, :],
                        w_sb[:, kc, h * 512 : (h + 1) * 512],
                        start=(kc == 0 and not cfg.has_bias),
                        stop=(kc == nch - 1),
                    )
            ob = opool.tile([128, d], BF16, tag="o")
            nc.scalar.activation(ob[:], ps2[:], mybir.ActivationFunctionType.Relu)
            # out-writes ride the scalar ring: the sync ring then only carries
            # DMA transposes (constant xbar mode, no per-block serialization).
            nc.scalar.dma_start(out_ap[blkno * 128 : (blkno + 1) * 128, :], ob[:])

        # Software pipeline: keep the final matmul of block b out of PE's
        # in-order queue until agg(b+LAG), so its flush+transpose are done.
        atts = {}
        for blkno in range(cfg.nblk):
            atts[blkno] = emit_agg(blkno)
            if blkno >= LAG:
                emit_final(blkno - LAG, atts.pop(blkno - LAG))
        for blkno in sorted(atts):
            emit_final(blkno, atts.pop(blkno))

    nc.compile()
    _prog_cache[cfg.key()] = nc
    return nc


def _run(cfg, per_core, xg, wmat, brow, trace=False):
    if trace:
        trace = _install_ntff_hook()
        if trace:
            import concourse.bass_utils as _bu

            _bu.upload_artifacts = lambda tmpdir: tmpdir  # no bucket in sandbox
    nc = _build(cfg)
    in_maps = []
    for k in range(N_CORES):
        idx_dev, p_dev = per_core[k]
        in_maps.append(
            {
                "xg": xg,
                "idx16": idx_dev,
                "ptil": p_dev,
                "wmat": wmat,
                "brow": brow,
            }
        )
    import tempfile

    tmpdir = tempfile.mkdtemp(prefix="bass_trace_") if trace else None
    res = run_bass_kernel_spmd(
        nc, in_maps, core_ids=list(range(N_CORES)), trace=trace, tmpdir=tmpdir
    )
    if trace:
        print(f"trace dir: {tmpdir}")
    global LAST_EXEC_NS, LAST_RESULTS
    LAST_EXEC_NS = res.exec_time_ns
    LAST_RESULTS = res
    out = np.concatenate([res.results[k]["out"] for k in range(N_CORES)], axis=0)
    return out


def kernel(**inputs):
    x = np.asarray(inputs["x"])
    cfg = _Cfg(x.shape[0], x.shape[1], T_TILE_DEFAULT, True)
    cfg, per_core, xg, wmat, brow = _prep(
        cfg,
        inputs["x"],
        inputs["edge_w"],
        inputs["W"],
        inputs["b"],
        inputs["src"],
        inputs["dst"],
    )
    out = _run(cfg, per_core, xg, wmat, brow, trace=TRACE)
    return np.ascontiguousarray(out.astype(np.float32))
